# revision 59
# baseline (speedup 1.0000x reference)
"""Multi-head self-attention (B=8, E=512, heads=8, S=1024) on 8 trn2 cores.

Sharding: data-parallel over batch — core b computes batch element b end to
end (no collectives).  Weights are replicated; all host-side prep is pure
data marshaling (transposes, dtype casts, power-of-2 scaling, fp8 hi/lo
splits) — every FLOP of the module runs on-device.

Key design points (all validated against the per-instruction cost model and
the compiled-NEFF execution):

  1. q/k/v projections run as fp8e4 DoubleRow matmuls with a hi/lo residual
     1.5-split (hi*hi + lo*hi + hi*lo; the lo*lo term is ~1e-3-relative and
     dropped).  DoubleRow contracts two 128-row k-blocks at 0.5 cycles/row,
     so the split costs 0.75x of bf16 while landing BETTER than bf16
     accuracy.  W carries a x16 scale to keep its residual out of the fp8
     subnormal range, compensated via x16 biases, vones=16 (so the softmax
     denominator scales identically) and the exp scale /256.
  2. Scores stay [keys, queries] in bf16 (K=64 per head, exp on ACT with the
     fold-in scale; no max-subtraction needed, |scores~bits| bounded).  The
     64 exps of [128,1024] are the ACT-chain spine (~66us) and run
     wall-to-wall; everything else is scheduled around keeping both the PE
     and this chain saturated.
  3. ctx for heads 0..6 is TOKEN-major: E[t2,t1] slices act as the
     stationary operand against v[t2, 65] (ones column accumulates the
     softmax denominator per token row), so all 128 output partitions are
     active (the channel-major form only fills 65).  Groups accumulate
     s-major — each group's 8 key-block matmuls run back-to-back and close
     before the next opens, because interleaving open accumulation groups
     within one PSUM bank corrupts all but the last-started group.  The 65-
     column groups live in a 14-slot rolling ring across 2 banks.
  4. Normalize (heads 0..6) is per-partition: reciprocal of the denominator
     columns + one strided broadcast-multiply into a pair-interleaved
     token-major block; PE transposes (via identity) flip each pair block
     back to channel-major zT through bf16-bitcast views of pj rotations.
  5. Head 7 runs the channel-major 65-partition ctx form so its ctx matmuls
     pipeline over key blocks (only the last waits on the final exp) and its
     normalized rows land directly in zT[3][64:128] — this keeps the
     program tail short.  Its normalize pipelines recip/broadcast/multiply
     at quarter (256-col) granularity.
  6. Output projection accumulates partials in SBUF as pairs complete
     ((0,1) seeded with the bias, (2) added); the finals fold the
     accumulator back into PSUM with an identity matmul and run on
     scavenged sc/pj/keeper tiles with one merged [128,1024] DMA per
     m-block (the DMA transfer engine is serial — fewer, larger transfers).
  7. The DMA transfer engine is serial and FIFO in issue order, and engine
     sequencers are in-order, so: all startup-critical DMAs ride the SP
     queue in exact need-order (an ACT-queue issue costs ~1.2us of ACT.SEQ
     and delays the exp dispatch stream); only the m=0 weight columns gate
     the first exp; v-units are scheduled after their weights land so they
     never block the in-order PE stream; p-state keeper matmuls (fp32, 4
     cy/row) chained off the normalize stages hold the PE clock across the
     tail's PE-idle window.

Measured: 88.9us (cost-model timeline), rel err 6.1e-3 vs the 2e-2 budget
(baseline at session start: 101.5us / 6.6e-3).
"""

import numpy as np
from contextlib import ExitStack

import concourse.bass as bass
import concourse.mybir as mybir
import concourse.tile as tile
from concourse import bacc
from concourse.bass_utils import run_bass_kernel_spmd

B = 8
C = 512
HH = 32
WW = 32
S = HH * WW            # 1024
HEADS = 8
HD = C // HEADS        # 64
CB = C // 128          # 4 channel blocks
TB = S // 128          # 8 token blocks
CHUNK = 512            # PSUM bank width in fp32
NCH = S // CHUNK       # 2
F32 = mybir.dt.float32
MM_DT = mybir.dt.bfloat16
F8 = mybir.dt.float8e4
WS = 16.0              # power-of-2 pre-scale on x-side fp8 projections
DR = mybir.MatmulPerfMode.DoubleRow

EXP = mybir.ActivationFunctionType.Exp
ADD = mybir.AluOpType.add
MULT = mybir.AluOpType.mult
BAND = mybir.AluOpType.bitwise_and
I16 = mybir.dt.int16
# corrected-Schraudolph constants (bits-domain exp with a parabola mantissa
# fix; 0.87% max / 0.30% rms, numpy-calibrated)
SCH_A = float(128.0 * np.log2(np.e) / 2048.0)
SCH_K = 2.655e-3


def build_nc(reps=1):
    nc = bacc.Bacc()
    # x8: [128, 2(hi/lo), 4(kblk), 1024(tok)] flattened; w8: q,k,v each as
    # (hi, lo) of W.T*16 in [128, 4(kblk), 512(oc)] layout, flattened in the
    # order (q_hi, q_lo, k_hi, k_lo, v_hi, v_lo).
    x8_d = nc.declare_dram_parameter("x8", [128, 2 * CB * S], F8, isOutput=False)
    w8_d = nc.declare_dram_parameter("w8", [128, 6 * CB * C], F8, isOutput=False)
    wo_d = nc.declare_dram_parameter("woT", [C, C], MM_DT, isOutput=False)
    ball_d = nc.declare_dram_parameter("ball", [C, 3], F32, isOutput=False)
    bvbc_d = nc.declare_dram_parameter("bv_bc", [128, C], F32, isOutput=False)
    vones_d = nc.declare_dram_parameter("vones", [128, 64], MM_DT, isOutput=False)
    ident_d = nc.declare_dram_parameter("ident", [128, 128], MM_DT, isOutput=False)
    out_d = nc.declare_dram_parameter("out", [C, S], MM_DT, isOutput=True)

    with tile.TileContext(nc) as tc, ExitStack() as ctx:
        pools = _make_pools(ctx, tc)
        for _ in range(reps):
            _emit(pools, nc, x8_d, w8_d, wo_d, ball_d, bvbc_d, vones_d, ident_d, out_d)
    nc.compile()
    return nc


def _make_pools(ctx, tc):
    return {
        "sb": ctx.enter_context(tc.tile_pool(name="sb", bufs=1)),
        "ps": ctx.enter_context(tc.tile_pool(name="ps", bufs=2, space="PSUM")),
        "ep": ctx.enter_context(tc.tile_pool(name="ep", bufs=13)),
        "np": ctx.enter_context(tc.tile_pool(name="npool", bufs=16)),
    }


def _emit(pools, nc, x8_d, w8_d, wo_d, ball_d, bvbc_d, vones_d, ident_d, out_d):
    # PSUM budget (8 banks): "sc" [128,1024] x2 = 4 banks (double-buffered
    # per-head score blocks), "ctx" [128,455] x2 = 2 banks (rolling ring of
    # 7-group ctx accumulators, token-major), "pj" [128,512] x2 = 2 banks
    # (projection / output-partial groups; transpose outputs ride free pj
    # rotations as bf16 bitcast views).
    sb = pools["sb"]
    ps = pools["ps"]
    ep = pools["ep"]
    np_pool = pools["np"]

    def sc_tile():
        return ps.tile([128, 1024], F32, tag="sc", bufs=2, name="sc")

    def ctx_slot_tile():
        return ps.tile([128, 7 * (HD + 1)], F32, tag="ctx", bufs=2, name="ctx")

    def pj_tile():
        return ps.tile([128, 512], F32, tag="pj", bufs=2, name="pj")

    # ---- SBUF tiles ----
    # fp8 hi/lo operands for the q/k/v projections (DoubleRow pairs over the
    # 4 contraction blocks).  x8 is [128, 2(hi/lo), 4(kblk), S]; each weight
    # piece is [128, 4(kblk), C].
    x8 = sb.tile([128, 2 * CB * S], F8, tag="x8", name="x8")
    xs8 = [x8[:, i * CB * S:(i + 1) * CB * S].rearrange("p (k t) -> p k t", k=CB)
           for i in range(2)]  # hi, lo — each [128, 4, 1024]
    w8 = sb.tile([128, 6 * CB * C], F8, tag="w8", name="w8")
    w = {n: tuple(
        w8[:, (2 * i + s) * CB * C:(2 * i + s + 1) * CB * C]
        .rearrange("p (k c) -> p k c", k=CB)
        for s in range(2))  # hi, lo — each [128, 4, 512]
        for i, n in enumerate(("wqT", "wkT", "wvT"))}
    w["woT"] = [sb.tile([128, C], MM_DT, tag=f"woT{j}", name=f"woT{j}")
                for j in range(CB)]
    ball = [sb.tile([128, 3], F32, tag=f"ball{m}", name=f"ball{m}") for m in range(CB)]
    bias = {n: [ball[m][:, i:i + 1] for m in range(CB)]
            for i, n in enumerate(("bq", "bk", "bo"))}
    bv_bc = sb.tile([128, C], F32, tag="bv_bc", name="bv_bc")
    ident = sb.tile([128, 128], MM_DT, tag="ident", name="ident")
    qT = [sb.tile([128, S], MM_DT, tag=f"qT{m}", name=f"qT{m}") for m in range(CB)]
    kT = [sb.tile([128, S], MM_DT, tag=f"kT{m}", name=f"kT{m}") for m in range(CB)]
    v = [sb.tile([128, HEADS * (HD + 1)], MM_DT, tag=f"v{i}", name=f"v{i}")
         for i in range(TB)]
    zT = [sb.tile([128, S], MM_DT, tag=f"zT{m}", name=f"zT{m}") for m in range(CB)]
    outacc = [[sb.tile([128, CHUNK], MM_DT, tag=f"oa{m}_{n}", name=f"oa{m}_{n}")
               for n in range(NCH)] for m in range(CB)]
    outT = [sb.tile([128, S], MM_DT, tag=f"ot{m}", name=f"ot{m}")
            for m in range(CB)]

    # ---- input DMAs ----
    # SP/HWDGE queue, ordered by first use: wq hi+lo and the first token-half
    # of x (hi then lo) so the very first projection chases the transfers,
    # then k's weights, the second token half, v's weights, and the
    # (late-needed) output-projection inputs.
    xd8 = [x8_d[:, i * CB * S:(i + 1) * CB * S].rearrange("p (k t) -> p k t", k=CB)
           for i in range(2)]
    # The DMA transfer engine is effectively serial, so the critical startup
    # prefix is kept minimal: only the m=0 columns (0:128) of the q/k weight
    # pieces plus the first token-half of x gate the first exp; everything
    # else streams behind.
    CC = CB * C
    wsb = [w8[:, i * CC:(i + 1) * CC].rearrange("p (k c) -> p k c", k=CB)
           for i in range(6)]
    wdd = [w8_d[:, i * CC:(i + 1) * CC].rearrange("p (k c) -> p k c", k=CB)
           for i in range(6)]
    # All startup-critical DMAs ride the SP queue in exact need-order (the
    # transfer engine is serial and FIFO in issue order; SP.SEQ has nothing
    # else to do, while an ACT-queue issue costs ~1.2us of ACT.SEQ time and
    # delays the exp dispatch stream).
    nc.sync.dma_start(wsb[0][:, :, 0:128], wdd[0][:, :, 0:128])   # q hi m0
    nc.sync.dma_start(xs8[0][:, :, 0:CHUNK], xd8[0][:, :, 0:CHUNK])
    nc.sync.dma_start(wsb[1][:, :, 0:128], wdd[1][:, :, 0:128])   # q lo m0
    nc.sync.dma_start(xs8[1][:, :, 0:CHUNK], xd8[1][:, :, 0:CHUNK])
    nc.sync.dma_start(wsb[2][:, :, 0:128], wdd[2][:, :, 0:128])   # k hi m0
    nc.sync.dma_start(wsb[3][:, :, 0:128], wdd[3][:, :, 0:128])   # k lo m0
    nc.sync.dma_start(xs8[0][:, :, CHUNK:S], xd8[0][:, :, CHUNK:S])
    nc.sync.dma_start(xs8[1][:, :, CHUNK:S], xd8[1][:, :, CHUNK:S])
    nc.sync.dma_start(w8[:, 4 * CC:6 * CC], w8_d[:, 4 * CC:6 * CC])  # v hi+lo
    nc.sync.dma_start(bv_bc, bvbc_d[:, :])
    v3 = [v[i].rearrange("p (h d) -> p h d", d=HD + 1) for i in range(TB)]
    for i in range(TB):
        nc.sync.dma_start(v3[i][:, :, HD:HD + 1], vones_d[:, 0:HEADS].unsqueeze(2))
    for pc in range(4):           # the rest of q/k
        nc.sync.dma_start(wsb[pc][:, :, 128:512], wdd[pc][:, :, 128:512])
    for j in range(CB):
        nc.sync.dma_start(w["woT"][j], wo_d[j * 128:(j + 1) * 128, :])
    nc.sync.dma_start(ident, ident_d[:, :])
    for m in range(1, CB):
        nc.sync.dma_start(ball[m], ball_d[m * 128:(m + 1) * 128, :])

    # ball0 rides the gpsimd SWDGE queue (needed early, tiny); the vones
    # columns go LAST on the SP queue so their descriptor-heavy transfers
    # never jump ahead of the critical startup prefix on the serial engine.
    nc.gpsimd.dma_start(ball[0], ball_d[0:128, :])

    # ---- PE work units ----
    # fp8 DoubleRow 1.5-split: hi*hi + lo*hi + hi*lo (the lo*lo term is
    # ~1e-3-relative and dropped).  Each DoubleRow matmul contracts a pair of
    # 128-row k-blocks at 0.5 cycles/row, so a unit costs 12 mms x 128 cy
    # vs bf16's 4 x 512.
    SPLIT = ((0, 0), (1, 0), (0, 1))  # (x piece, w piece)

    def qk_unit(wn, bn, dest, m, n, mid=None, drain=None):
        pt = pj_tile()
        for nh in range(2):
            nsl = slice(n * CHUNK + nh * 256, n * CHUNK + (nh + 1) * 256)
            osl = slice(nh * 256, (nh + 1) * 256)
            for ti, (xi, wi) in enumerate(SPLIT):
                for j2 in range(CB // 2):
                    nc.tensor.matmul(
                        pt[:, osl],
                        lhsT=w[wn][wi][:, 2 * j2:2 * j2 + 2, m * 128:(m + 1) * 128],
                        rhs=xs8[xi][:, 2 * j2:2 * j2 + 2, nsl],
                        start=(ti == 0 and j2 == 0),
                        stop=(ti == 2 and j2 == 1),
                        perf_mode=DR,
                    )
            if mid is not None and nh == 0:
                mid()
        if drain is not None:
            drain(dest[m][:, n * CHUNK:(n + 1) * CHUNK], pt, bias[bn][m])
        else:
            nc.vector.tensor_scalar_add(
                dest[m][:, n * CHUNK:(n + 1) * CHUNK], pt[:, 0:512], bias[bn][m]
            )

    def v_unit(t2, mid=None):
        pt = pj_tile()
        tsl = slice(t2 * 128, (t2 + 1) * 128)
        for nh in range(2):
            osl = slice(nh * 256, (nh + 1) * 256)
            for ti, (xi, wi) in enumerate(SPLIT):
                for j2 in range(CB // 2):
                    nc.tensor.matmul(
                        pt[:, osl],
                        lhsT=xs8[xi][:, 2 * j2:2 * j2 + 2, tsl],
                        rhs=w["wvT"][wi][:, 2 * j2:2 * j2 + 2, osl],
                        start=(ti == 0 and j2 == 0),
                        stop=(ti == 2 and j2 == 1),
                        perf_mode=DR,
                    )
            if mid is not None and nh == 0:
                mid()
        nc.vector.tensor_tensor(
            v3[t2][:, :, 0:HD],
            pt[:, 0:512].rearrange("p (h d) -> p h d", d=HD),
            bv_bc.rearrange("p (h d) -> p h d", d=HD),
            ADD,
        )

    def o_unit(js, m, n, seed=False, final=False, pt=None, swdge=False,
               drain=None):
        # output projection partial over K blocks `js` (pairs), accumulated
        # in SBUF (seed carries the bias).  A final unit folds the SBUF
        # accumulator back into the PSUM group with an identity matmul (PE,
        # cheap) so the drain is a plain copy on whichever engine has slack.
        if pt is None:
            pt = pj_tile()
        if final:
            nc.tensor.matmul(pt[:, 0:512], lhsT=ident, rhs=outacc[m][n],
                             start=True, stop=False)
        for i, j in enumerate(js):
            nc.tensor.matmul(
                pt[:, 0:512],
                lhsT=w["woT"][j][:, m * 128:(m + 1) * 128],
                rhs=zT[j][:, n * CHUNK:(n + 1) * CHUNK],
                start=(not final and i == 0),
                stop=(i == len(js) - 1),
            )
        if final and drain is not False:
            (drain or nc.vector.tensor_copy)(
                outT[m][:, n * CHUNK:(n + 1) * CHUNK], pt[:, 0:512])
        elif seed:
            nc.vector.tensor_scalar_add(outacc[m][n], pt[:, 0:512], bias["bo"][m])
        else:
            nc.vector.tensor_tensor(outacc[m][n], pt[:, 0:512], outacc[m][n], ADD)

    # ---- token-major attention stream ----
    # Scores stay [keys, queries]; exp tiles E[t2, t1] then act as the
    # stationary operand of the ctx matmuls, so ctx lands token-major
    # [t1, dv] with all 128 output partitions active (the channel-major form
    # only fills 65).  Column 64 of each ctx group accumulates the softmax
    # denominator via the ones column in v.  After both heads of a pair are
    # normalized into a shared [t1, dv-pair] block, PE transposes flip it to
    # channel-major zT for the (unchanged) output projection.
    ESC = float(1.0 / np.sqrt(HD) / (WS * WS))
    GW = HD + 1            # ctx group width (64 dv + denominator)
    slots = {}

    def ctx_group(g):
        k = g // 7
        if k not in slots:
            slots[k] = ctx_slot_tile()
        c = (g % 7) * GW
        return slots[k][:, c:c + GW]

    E_store = {}
    # a few exps run on the DVE via corrected Schraudolph so the saturated
    # ACT exp chain (the kernel's spine) shortens; sites are picked away
    # from normalize/transpose steps so the DVE queue stays clear.
    # Corrected-Schraudolph DVE exp (see dve_exp) is numerically good
    # (0.3% rms) but measured net-negative at any site: its ~3.3us of
    # consecutive DVE ops delays the in-order drain queue that feeds the
    # PE stream.  Left disabled; would need per-step op interleaving.
    DVE_EXP = set()
    mask127 = np_pool.tile([128, 1024], I16, tag="m127", bufs=1, name="m127")
    nc.gpsimd.memset(mask127, 127)

    def dve_exp(E, sc):
        bs = np_pool.tile([128, 1024], I16, tag="xbs", bufs=1, name="xbs")
        mu = np_pool.tile([128, 1024], I16, tag="xmu", bufs=1, name="xmu")
        mC = np_pool.tile([128, 1024], MM_DT, tag="xmc", bufs=1, name="xmc")
        w = np_pool.tile([128, 1024], MM_DT, tag="xw", bufs=1, name="xw")
        nc.vector.tensor_scalar(bs, sc, SCH_A, 16256.0, MULT, ADD)
        nc.vector.tensor_tensor(mu, bs, mask127, BAND)
        nc.vector.tensor_scalar(mC, mu, SCH_K, None, MULT)
        nc.vector.scalar_tensor_tensor(w, mu, -128.0, mC, ADD, MULT)
        nc.vector.tensor_tensor(E.bitcast(I16), bs, w, ADD)

    def scores_exp(h, t2):
        hp, par = divmod(h, 2)
        rows = slice(par * 64, (par + 1) * 64)
        sc = sc_tile()
        for half in range(2):
            nc.tensor.matmul(
                sc[:, half * 512:(half + 1) * 512],
                lhsT=kT[hp][rows, t2 * 128:(t2 + 1) * 128],
                rhs=qT[hp][rows, half * 512:(half + 1) * 512],
                start=True, stop=True,
                tile_position=(par * 64, 0),
            )
        E = ep.tile([128, 1024], MM_DT, tag="E", name="E")
        if (h, t2) in DVE_EXP:
            dve_exp(E, sc)
        else:
            nc.scalar.activation(E, sc, EXP, scale=ESC)
        E_store[(h, t2)] = E

    def ctx_burst(h, s0, s1):
        # s-major: each ctx group's 8 accumulation matmuls run back-to-back
        # and the group closes before the next opens — interleaving open
        # accumulation groups within one PSUM bank corrupts all but the
        # last-started group (hardware pending-zero granularity).
        for s in range(s0, s1):
            grp = ctx_group(8 * h + s)
            for t2 in range(TB):
                nc.tensor.matmul(
                    grp,
                    lhsT=E_store[(h, t2)][:, s * 128:(s + 1) * 128],
                    rhs=v[t2][:, h * GW:(h + 1) * GW],
                    start=(t2 == 0), stop=(t2 == TB - 1),
                )
        if s1 == 8:
            for t2 in range(TB):
                E_store.pop((h, t2))

    ztoks = {}

    def ztok_tile(p):
        if p not in ztoks:
            ztoks[p] = np_pool.tile([128, S], MM_DT, tag="ztok", bufs=2,
                                    name="ztok")
        return ztoks[p]

    def norm_unit(h):
        # reciprocal of the 8 denominator columns, then one strided multiply
        # per slot-run into the pair's token-major block (per-partition
        # scalars: no gpsimd broadcast needed in this orientation).
        par = h % 2
        ztr = ztok_tile(h // 2).rearrange("p (s q) -> p s q", q=128)
        rs = np_pool.tile([128, 8], F32, tag="rs", bufs=2, name="rs")
        rsr = rs.rearrange("p (g o) -> p g o", o=1)
        g0 = 8 * h
        views = []
        b = g0
        while b < g0 + 8:
            e = min(g0 + 8, (b // 7 + 1) * 7)
            kr = slots[b // 7].rearrange("p (g c) -> p g c", c=GW)
            i0 = b % 7
            views.append((b - g0, e - b, kr[:, i0:i0 + e - b, :]))
            b = e
        for s0, n, kr in views:
            nc.vector.reciprocal(rsr[:, s0:s0 + n, :], kr[:, :, HD:HD + 1])
        for s0, n, kr in views:
            nc.vector.tensor_tensor(
                ztr[:, s0:s0 + n, par * 64:(par + 1) * 64],
                kr[:, :, 0:HD],
                rsr[:, s0:s0 + n, :].broadcast_to([128, n, HD]),
                MULT,
            )

    def tr_unit(p, half, head_only=False):
        # 4 PE transposes of [t1-slice, dv-pair] blocks into a bf16 view of a
        # pj rotation slot, drained as one [128, 512] copy into zT[p].
        # head_only: pair 3's odd head runs channel-major (65-partition ctx),
        # so only head 6's 64 dv columns go through the transpose — and since
        # cx7 owns the pj rotations by then, pair 3 borrows a ctx-ring bank.
        trv = (ctx_slot_tile() if head_only else pj_tile()).bitcast(MM_DT)
        ztp = ztok_tile(p)
        wdt = 64 if head_only else 128
        for sl in range(4):
            s = half * 4 + sl
            nc.tensor.matmul(
                trv[0:wdt, sl * 128:(sl + 1) * 128],
                lhsT=ztp[:, s * 128:s * 128 + wdt],
                rhs=ident, is_transpose=True,
            )
        nc.vector.tensor_copy(zT[p][0:wdt, half * 512:(half + 1) * 512],
                              trv[0:wdt, 0:512])

    # head 7 runs the channel-major 65-partition ctx form so its ctx matmuls
    # pipeline over t2 (only the last block waits on the final exp) and the
    # normalized rows land directly in zT[3][64:128] — this keeps the program
    # tail short.  cx7 comes from the last two pj rotations; no pj user may
    # follow until the output-projection finals (which scavenge sc tiles).
    cx7 = []

    def ctx65(t2):
        if not cx7:
            cx7.extend((pj_tile(), pj_tile()))
        for half in range(2):
            nc.tensor.matmul(
                cx7[half][0:HD + 1, :],
                lhsT=v[t2][:, 7 * GW:8 * GW],
                rhs=E_store[(7, t2)][:, half * 512:(half + 1) * 512],
                start=(t2 == 0), stop=(t2 == TB - 1),
            )
        if t2 == TB - 1:
            for tt in range(TB):
                E_store.pop((7, tt))

    # ---- emission schedule ----
    # Step sigma = 8h + t2.  Fixed slots by sigma%8: +4..+7 carry ctx batches
    # (h, t2-4); +0/+1 carry the previous head's late ctx batches (lag
    # shrinks to 2 for t2=7 so normalize lands early enough for the 14-slot
    # ctx ring); +2 normalizes the previous head.  tr units ride the +3/+4
    # slots after odd heads.  Everything else (projections, output-partial
    # units) fills the remaining slots.
    F = lambda f, *a, **k: (lambda: f(*a, **k))

    # startup: the minimal chain to the first exp is q00+k00 -> scores(0,0)
    # half A; q01 and v0 ride between the two half-exps.
    IDENT_F = mybir.ActivationFunctionType.Identity

    def act_drain(dest, pt, b):
        nc.scalar.activation(dest, pt[:, 0:512], IDENT_F, bias=b)

    qk_unit("wqT", "bq", qT, 0, 0)
    qk_unit("wkT", "bk", kT, 0, 0, drain=act_drain)
    sc0 = sc_tile()
    E0 = ep.tile([128, 1024], MM_DT, tag="E", name="E")
    nc.tensor.matmul(sc0[:, 0:512], lhsT=kT[0][0:64, 0:128],
                     rhs=qT[0][0:64, 0:512], start=True, stop=True,
                     tile_position=(0, 0))
    nc.scalar.activation(E0[:, 0:512], sc0[:, 0:512], EXP, scale=ESC)
    qk_unit("wqT", "bq", qT, 0, 1)
    # scores(0,1) half A needs only q00+k00 — its exp fills the ACT gap
    # while the B halves wait on q01's drain (x2-DMA-gated).
    sc1 = sc_tile()
    E1 = ep.tile([128, 1024], MM_DT, tag="E", name="E")
    nc.tensor.matmul(sc1[:, 0:512], lhsT=kT[0][0:64, 128:256],
                     rhs=qT[0][0:64, 0:512], start=True, stop=True,
                     tile_position=(0, 0))
    nc.scalar.activation(E1[:, 0:512], sc1[:, 0:512], EXP, scale=ESC)
    nc.tensor.matmul(sc0[:, 512:1024], lhsT=kT[0][0:64, 0:128],
                     rhs=qT[0][0:64, 512:1024], start=True, stop=True,
                     tile_position=(0, 0))
    nc.scalar.activation(E0[:, 512:1024], sc0[:, 512:1024], EXP, scale=ESC)
    nc.tensor.matmul(sc1[:, 512:1024], lhsT=kT[0][0:64, 128:256],
                     rhs=qT[0][0:64, 512:1024], start=True, stop=True,
                     tile_position=(0, 0))
    nc.scalar.activation(E1[:, 512:1024], sc1[:, 512:1024], EXP, scale=ESC)
    E_store[(0, 0)] = E0
    E_store[(0, 1)] = E1
    qk_unit("wkT", "bk", kT, 0, 1)

    fillers = {
        4: [F(v_unit, 0)], 5: [F(v_unit, 1), F(v_unit, 2)],
        6: [F(v_unit, 3), F(v_unit, 4)], 7: [F(v_unit, 5), F(v_unit, 6)],
        8: [F(v_unit, 7)],
        12: [F(qk_unit, "wqT", "bq", qT, 1, 0)],
        13: [F(qk_unit, "wkT", "bk", kT, 1, 0)],
        14: [F(qk_unit, "wqT", "bq", qT, 1, 1)],
        15: [F(qk_unit, "wkT", "bk", kT, 1, 1)],
        22: [F(tr_unit, 0, 0)],
        23: [F(tr_unit, 0, 1)],
        24: [F(qk_unit, "wqT", "bq", qT, 2, 0)],
        25: [F(qk_unit, "wkT", "bk", kT, 2, 0)],
        26: [F(qk_unit, "wqT", "bq", qT, 2, 1)],
        27: [F(qk_unit, "wkT", "bk", kT, 2, 1)],
        33: [F(qk_unit, "wqT", "bq", qT, 3, 0)],
        34: [F(qk_unit, "wkT", "bk", kT, 3, 0)],
        35: [F(qk_unit, "wqT", "bq", qT, 3, 1)],
        36: [F(qk_unit, "wkT", "bk", kT, 3, 1)],
        38: [F(tr_unit, 1, 0)],
        39: [F(tr_unit, 1, 1)],
        41: [F(o_unit, (0, 1), 0, 0, True)],
        42: [F(o_unit, (0, 1), 1, 0, True)],
        43: [F(o_unit, (0, 1), 2, 0, True)],
        44: [F(o_unit, (0, 1), 3, 0, True)],
        45: [F(o_unit, (0, 1), 0, 1, True)],
        46: [F(o_unit, (0, 1), 1, 1, True)],
        47: [F(o_unit, (0, 1), 2, 1, True)],
        48: [F(o_unit, (0, 1), 3, 1, True)],
        54: [F(tr_unit, 2, 0)],
        55: [F(tr_unit, 2, 1)],
        # pj users must all precede sig 60 where ctx65 claims cx7; pair 3's
        # transposes borrow the ctx ring instead.
        56: [F(o_unit, (2,), 0, 0), F(o_unit, (2,), 1, 0)],
        57: [F(o_unit, (2,), 2, 0), F(o_unit, (2,), 3, 0)],
        58: [F(o_unit, (2,), 0, 1), F(o_unit, (2,), 1, 1)],
        59: [F(o_unit, (2,), 2, 1), F(o_unit, (2,), 3, 1)],
        62: [F(tr_unit, 3, 0, True)],
        63: [F(tr_unit, 3, 1, True)],
    }

    for sig in range(1, 64):
        h, t2 = divmod(sig, 8)
        if sig != 1:          # (0,1) was emitted half-split in the startup
            scores_exp(h, t2)
        if h > 0 and 1 <= t2 <= 4:
            ctx_burst(h - 1, 2 * (t2 - 1), 2 * (t2 - 1) + 2)
        elif h > 0 and t2 == 5:
            norm_unit(h - 1)
        if h == 7 and t2 >= 4:
            ctx65(t2 - 4)
        for f in fillers.get(sig, ()):
            f()

    # ---- tail ----
    # head 7's trailing ctx65 blocks chase the last exps; its normalize goes
    # through the channel-major path (recip row + gpsimd broadcast) straight
    # into zT[3][64:128].  Keepers hold the PE p-state across the vector-side
    # latency; the finals scavenge sc tiles (pj holds cx7).
    ctx65(4)
    ctx65(5)
    ctx65(6)
    ctx65(7)
    # quarter-granular normalize pipeline: recip -> broadcast -> multiply
    # flows per [*, 256] chunk so the first zT quarters land early.
    rss, rbs = [], []
    for q in range(4):
        rs = np_pool.tile([1, 256], F32, tag="rs7", name="rs7")
        nc.vector.reciprocal(
            rs[0:1, :], cx7[q // 2][64:65, (q % 2) * 256:(q % 2 + 1) * 256])
        rss.append(rs)
        rb = np_pool.tile([64, 256], F32, tag="rb7", name="rb7")
        nc.gpsimd.partition_broadcast(rb, rs[0:1, :], 64)
        rbs.append(rb)
    # keepers: chained off the normalize stages so the PE p-state holds
    # across the whole PE-idle window (fp32 operands run 4 cy/row — good).
    warm = sc_tile()
    nc.tensor.matmul(warm[:, 0:512], lhsT=ident, rhs=qT[0][:, 0:512],
                     start=True, stop=True)
    nc.tensor.matmul(warm[0:64, 512:768], lhsT=bv_bc[0:64, 0:64],
                     rhs=rbs[0][:, 0:256], start=True, stop=True)
    nc.tensor.matmul(warm[0:64, 768:1024], lhsT=bv_bc[0:64, 0:64],
                     rhs=rbs[1][:, 0:256], start=True, stop=True)
    for q in range(4):
        nc.vector.tensor_tensor(
            zT[3][64:128, q * 256:(q + 1) * 256],
            cx7[q // 2][0:64, (q % 2) * 256:(q % 2 + 1) * 256], rbs[q], MULT)
    # finals: 2 per scavenged sc tile, drains alternating ACT/DVE, one merged
    # [128, 1024] output DMA per m-block (the transfer engine is serial, so
    # fewer, larger transfers shorten the issue pipeline).
    # finals: per-tile units (accumulator fold first inside o_unit), homes
    # chosen so WAR chains resolve during the normalize window.
    drains = (nc.scalar.copy, nc.vector.tensor_copy)
    homes = {0: None, 1: None, 2: "pj", 3: "warm"}
    for m in range(CB):
        if homes[m] == "pj":
            pts = [pj_tile() for _ in range(NCH)]
        elif homes[m] == "warm":
            pts = [warm[:, n * 512:(n + 1) * 512] for n in range(NCH)]
        else:
            sct = sc_tile()
            pts = [sct[:, n * 512:(n + 1) * 512] for n in range(NCH)]
        for n in range(NCH):
            o_unit((3,), m, n, final=True, pt=pts[n],
                   drain=drains[(2 * m + n) % 2])
        (nc.sync if m % 2 == 0 else nc.gpsimd).dma_start(
            out_d[m * 128:(m + 1) * 128, :], outT[m])


_NC_CACHE = None


def _get_nc():
    global _NC_CACHE
    if _NC_CACHE is None:
        _NC_CACHE = build_nc()
    return _NC_CACHE


def _split_f8(a, f8):
    """fp8 hi/lo split: a ~= hi + lo elementwise."""
    hi = a.astype(f8)
    lo = (a - hi.astype(np.float32)).astype(f8)
    return hi, lo


def _blk(a):
    """[512, N] -> [128, 4, N] -> [128, 4*N] (contraction-block-major)."""
    n = a.shape[1]
    return np.ascontiguousarray(
        a.reshape(CB, 128, n).transpose(1, 0, 2).reshape(128, CB * n)
    )


def _in_maps(x, Wq, bq, Wk, bk, Wv, bv, Wo, bo):
    x = np.ascontiguousarray(np.asarray(x, np.float32))
    bf16 = mybir.dt.np(MM_DT)
    f8 = mybir.dt.np(F8)
    # q/k/v weights carry a x16 scale (keeps W and its fp8 residual out of
    # the subnormal range); compensated by x16 biases, vones=16 (so the
    # softmax denominator scales identically) and the exp scale /256.
    wsplit = [_split_f8(np.asarray(W, np.float32).T * WS, f8) for W in (Wq, Wk, Wv)]
    w8 = np.concatenate([_blk(p) for hl in wsplit for p in hl], axis=1)
    base = {
        "bv_bc": np.ascontiguousarray(
            np.broadcast_to(np.asarray(bv, np.float32) * WS, (128, C))
        ),
        "w8": w8,
        "woT": np.ascontiguousarray(np.asarray(Wo, np.float32).T.astype(bf16)),
        "ball": np.ascontiguousarray(np.stack(
            [np.asarray(bq, np.float32) * WS, np.asarray(bk, np.float32) * WS,
             np.asarray(bo, np.float32)], axis=1)),
        "vones": np.full((128, 64), WS, bf16),
        "ident": np.eye(128, dtype=bf16),
    }

    def xmap(b):
        xsT = x[b].reshape(S, C).T  # [C, S] channel-major, f32
        hi, lo = _split_f8(xsT, f8)
        return np.concatenate([_blk(hi), _blk(lo)], axis=1)

    return [dict(base, x8=xmap(b)) for b in range(B)]


def _run(trace=False, **inputs):
    nc = _get_nc()
    maps = _in_maps(**inputs)
    res = run_bass_kernel_spmd(nc, maps, core_ids=list(range(B)), trace=trace)
    out = np.stack(
        [np.asarray(res.results[b]["out"]).reshape(C, HH, WW) for b in range(B)]
    ).astype(np.float32)
    return out, res


def kernel(**inputs):
    out, _ = _run(trace=False, **inputs)
    return out



# revision 60
# speedup vs baseline: 1.0051x; 1.0051x over previous
"""Multi-head self-attention (B=8, E=512, heads=8, S=1024) on 8 trn2 cores.

Sharding: data-parallel over batch — core b computes batch element b end to
end (no collectives).  Weights are replicated; all host-side prep is pure
data marshaling (transposes, dtype casts, power-of-2 scaling, fp8 hi/lo
splits) — every FLOP of the module runs on-device.

Key design points (all validated against the per-instruction cost model and
the compiled-NEFF execution):

  1. q/k/v projections run as fp8e4 DoubleRow matmuls with a hi/lo residual
     1.5-split (hi*hi + lo*hi + hi*lo; the lo*lo term is ~1e-3-relative and
     dropped).  DoubleRow contracts two 128-row k-blocks at 0.5 cycles/row,
     so the split costs 0.75x of bf16 while landing BETTER than bf16
     accuracy.  W carries a x16 scale to keep its residual out of the fp8
     subnormal range, compensated via x16 biases, vones=16 (so the softmax
     denominator scales identically) and the exp scale /256.
  2. Scores stay [keys, queries] in bf16 (K=64 per head, exp on ACT with the
     fold-in scale; no max-subtraction needed, |scores~bits| bounded).  The
     64 exps of [128,1024] are the ACT-chain spine (~66us) and run
     wall-to-wall; everything else is scheduled around keeping both the PE
     and this chain saturated.
  3. ctx for heads 0..6 is TOKEN-major: E[t2,t1] slices act as the
     stationary operand against v[t2, 65] (ones column accumulates the
     softmax denominator per token row), so all 128 output partitions are
     active (the channel-major form only fills 65).  Groups accumulate
     s-major — each group's 8 key-block matmuls run back-to-back and close
     before the next opens, because interleaving open accumulation groups
     within one PSUM bank corrupts all but the last-started group.  The 65-
     column groups live in a 14-slot rolling ring across 2 banks.
  4. Normalize (heads 0..6) is per-partition: reciprocal of the denominator
     columns + one strided broadcast-multiply into a pair-interleaved
     token-major block; PE transposes (via identity) flip each pair block
     back to channel-major zT through bf16-bitcast views of pj rotations.
  5. Head 7 runs the channel-major 65-partition ctx form so its ctx matmuls
     pipeline over key blocks (only the last waits on the final exp) and its
     normalized rows land directly in zT[3][64:128] — this keeps the
     program tail short.  Its normalize pipelines recip/broadcast/multiply
     at quarter (256-col) granularity.
  6. Output projection accumulates partials in SBUF as pairs complete
     ((0,1) seeded with the bias, (2) added); the finals fold the
     accumulator back into PSUM with an identity matmul and run on
     scavenged sc/pj/keeper tiles with one merged [128,1024] DMA per
     m-block (the DMA transfer engine is serial — fewer, larger transfers).
  7. The DMA transfer engine is serial and FIFO in issue order, and engine
     sequencers are in-order, so: all startup-critical DMAs ride the SP
     queue in exact need-order (an ACT-queue issue costs ~1.2us of ACT.SEQ
     and delays the exp dispatch stream); only the m=0 weight columns gate
     the first exp; v-units are scheduled after their weights land so they
     never block the in-order PE stream; p-state keeper matmuls (fp32, 4
     cy/row) chained off the normalize stages hold the PE clock across the
     tail's PE-idle window.

Measured: 88.8us (cost-model timeline), rel err 6.1e-3 vs the 2e-2 budget
(baseline at session start: 101.5us / 6.6e-3).
"""

import numpy as np
from contextlib import ExitStack

import concourse.bass as bass
import concourse.mybir as mybir
import concourse.tile as tile
from concourse import bacc
from concourse.bass_utils import run_bass_kernel_spmd

B = 8
C = 512
HH = 32
WW = 32
S = HH * WW            # 1024
HEADS = 8
HD = C // HEADS        # 64
CB = C // 128          # 4 channel blocks
TB = S // 128          # 8 token blocks
CHUNK = 512            # PSUM bank width in fp32
NCH = S // CHUNK       # 2
F32 = mybir.dt.float32
MM_DT = mybir.dt.bfloat16
F8 = mybir.dt.float8e4
WS = 16.0              # power-of-2 pre-scale on x-side fp8 projections
DR = mybir.MatmulPerfMode.DoubleRow

EXP = mybir.ActivationFunctionType.Exp
ADD = mybir.AluOpType.add
MULT = mybir.AluOpType.mult
BAND = mybir.AluOpType.bitwise_and
I16 = mybir.dt.int16
# corrected-Schraudolph constants (bits-domain exp with a parabola mantissa
# fix; 0.87% max / 0.30% rms, numpy-calibrated)
SCH_A = float(128.0 * np.log2(np.e) / 2048.0)
SCH_K = 2.655e-3


def build_nc(reps=1):
    nc = bacc.Bacc()
    # x8: [128, 2(hi/lo), 4(kblk), 1024(tok)] flattened; w8: q,k,v each as
    # (hi, lo) of W.T*16 in [128, 4(kblk), 512(oc)] layout, flattened in the
    # order (q_hi, q_lo, k_hi, k_lo, v_hi, v_lo).
    x8_d = nc.declare_dram_parameter("x8", [128, 2 * CB * S], F8, isOutput=False)
    w8_d = nc.declare_dram_parameter("w8", [128, 6 * CB * C], F8, isOutput=False)
    wo_d = nc.declare_dram_parameter("woT", [C, C], MM_DT, isOutput=False)
    ball_d = nc.declare_dram_parameter("ball", [C, 3], F32, isOutput=False)
    bvbc_d = nc.declare_dram_parameter("bv_bc", [128, C], F32, isOutput=False)
    vones_d = nc.declare_dram_parameter("vones", [128, 64], MM_DT, isOutput=False)
    ident_d = nc.declare_dram_parameter("ident", [128, 128], MM_DT, isOutput=False)
    out_d = nc.declare_dram_parameter("out", [C, S], MM_DT, isOutput=True)

    with tile.TileContext(nc) as tc, ExitStack() as ctx:
        pools = _make_pools(ctx, tc)
        for _ in range(reps):
            _emit(pools, nc, x8_d, w8_d, wo_d, ball_d, bvbc_d, vones_d, ident_d, out_d)
    nc.compile()
    return nc


def _make_pools(ctx, tc):
    return {
        "sb": ctx.enter_context(tc.tile_pool(name="sb", bufs=1)),
        "ps": ctx.enter_context(tc.tile_pool(name="ps", bufs=2, space="PSUM")),
        "ep": ctx.enter_context(tc.tile_pool(name="ep", bufs=13)),
        "np": ctx.enter_context(tc.tile_pool(name="npool", bufs=16)),
    }


def _emit(pools, nc, x8_d, w8_d, wo_d, ball_d, bvbc_d, vones_d, ident_d, out_d):
    # PSUM budget (8 banks): "sc" [128,1024] x2 = 4 banks (double-buffered
    # per-head score blocks), "ctx" [128,455] x2 = 2 banks (rolling ring of
    # 7-group ctx accumulators, token-major), "pj" [128,512] x2 = 2 banks
    # (projection / output-partial groups; transpose outputs ride free pj
    # rotations as bf16 bitcast views).
    sb = pools["sb"]
    ps = pools["ps"]
    ep = pools["ep"]
    np_pool = pools["np"]

    def sc_tile():
        return ps.tile([128, 1024], F32, tag="sc", bufs=2, name="sc")

    def ctx_slot_tile():
        return ps.tile([128, 7 * (HD + 1)], F32, tag="ctx", bufs=2, name="ctx")

    def pj_tile():
        return ps.tile([128, 512], F32, tag="pj", bufs=2, name="pj")

    # ---- SBUF tiles ----
    # fp8 hi/lo operands for the q/k/v projections (DoubleRow pairs over the
    # 4 contraction blocks).  x8 is [128, 2(hi/lo), 4(kblk), S]; each weight
    # piece is [128, 4(kblk), C].
    x8 = sb.tile([128, 2 * CB * S], F8, tag="x8", name="x8")
    xs8 = [x8[:, i * CB * S:(i + 1) * CB * S].rearrange("p (k t) -> p k t", k=CB)
           for i in range(2)]  # hi, lo — each [128, 4, 1024]
    w8 = sb.tile([128, 6 * CB * C], F8, tag="w8", name="w8")
    w = {n: tuple(
        w8[:, (2 * i + s) * CB * C:(2 * i + s + 1) * CB * C]
        .rearrange("p (k c) -> p k c", k=CB)
        for s in range(2))  # hi, lo — each [128, 4, 512]
        for i, n in enumerate(("wqT", "wkT", "wvT"))}
    w["woT"] = [sb.tile([128, C], MM_DT, tag=f"woT{j}", name=f"woT{j}")
                for j in range(CB)]
    ball = [sb.tile([128, 3], F32, tag=f"ball{m}", name=f"ball{m}") for m in range(CB)]
    bias = {n: [ball[m][:, i:i + 1] for m in range(CB)]
            for i, n in enumerate(("bq", "bk", "bo"))}
    bv_bc = sb.tile([128, C], F32, tag="bv_bc", name="bv_bc")
    ident = sb.tile([128, 128], MM_DT, tag="ident", name="ident")
    qT = [sb.tile([128, S], MM_DT, tag=f"qT{m}", name=f"qT{m}") for m in range(CB)]
    kT = [sb.tile([128, S], MM_DT, tag=f"kT{m}", name=f"kT{m}") for m in range(CB)]
    v = [sb.tile([128, HEADS * (HD + 1)], MM_DT, tag=f"v{i}", name=f"v{i}")
         for i in range(TB)]
    zT = [sb.tile([128, S], MM_DT, tag=f"zT{m}", name=f"zT{m}") for m in range(CB)]
    outacc = [[sb.tile([128, CHUNK], MM_DT, tag=f"oa{m}_{n}", name=f"oa{m}_{n}")
               for n in range(NCH)] for m in range(CB)]
    outT = [sb.tile([128, S], MM_DT, tag=f"ot{m}", name=f"ot{m}")
            for m in range(CB)]

    # ---- input DMAs ----
    # SP/HWDGE queue, ordered by first use: wq hi+lo and the first token-half
    # of x (hi then lo) so the very first projection chases the transfers,
    # then k's weights, the second token half, v's weights, and the
    # (late-needed) output-projection inputs.
    xd8 = [x8_d[:, i * CB * S:(i + 1) * CB * S].rearrange("p (k t) -> p k t", k=CB)
           for i in range(2)]
    # The DMA transfer engine is effectively serial, so the critical startup
    # prefix is kept minimal: only the m=0 columns (0:128) of the q/k weight
    # pieces plus the first token-half of x gate the first exp; everything
    # else streams behind.
    CC = CB * C
    wsb = [w8[:, i * CC:(i + 1) * CC].rearrange("p (k c) -> p k c", k=CB)
           for i in range(6)]
    wdd = [w8_d[:, i * CC:(i + 1) * CC].rearrange("p (k c) -> p k c", k=CB)
           for i in range(6)]
    # All startup-critical DMAs ride the SP queue in exact need-order (the
    # transfer engine is serial and FIFO in issue order; SP.SEQ has nothing
    # else to do, while an ACT-queue issue costs ~1.2us of ACT.SEQ time and
    # delays the exp dispatch stream).
    nc.sync.dma_start(wsb[0][:, :, 0:128], wdd[0][:, :, 0:128])   # q hi m0
    nc.sync.dma_start(xs8[0][:, :, 0:CHUNK], xd8[0][:, :, 0:CHUNK])
    nc.sync.dma_start(wsb[1][:, :, 0:128], wdd[1][:, :, 0:128])   # q lo m0
    nc.sync.dma_start(xs8[1][:, :, 0:CHUNK], xd8[1][:, :, 0:CHUNK])
    nc.sync.dma_start(wsb[2][:, :, 0:128], wdd[2][:, :, 0:128])   # k hi m0
    nc.sync.dma_start(wsb[3][:, :, 0:128], wdd[3][:, :, 0:128])   # k lo m0
    nc.sync.dma_start(xs8[0][:, :, CHUNK:S], xd8[0][:, :, CHUNK:S])
    nc.sync.dma_start(xs8[1][:, :, CHUNK:S], xd8[1][:, :, CHUNK:S])
    nc.sync.dma_start(w8[:, 4 * CC:6 * CC], w8_d[:, 4 * CC:6 * CC])  # v hi+lo
    nc.sync.dma_start(bv_bc, bvbc_d[:, :])
    v3 = [v[i].rearrange("p (h d) -> p h d", d=HD + 1) for i in range(TB)]
    for i in range(TB):
        nc.sync.dma_start(v3[i][:, :, HD:HD + 1], vones_d[:, 0:HEADS].unsqueeze(2))
    for pc in range(4):           # the rest of q/k
        nc.sync.dma_start(wsb[pc][:, :, 128:512], wdd[pc][:, :, 128:512])
    for j in range(CB):
        nc.sync.dma_start(w["woT"][j], wo_d[j * 128:(j + 1) * 128, :])
    nc.sync.dma_start(ident, ident_d[:, :])
    for m in range(1, CB):
        nc.sync.dma_start(ball[m], ball_d[m * 128:(m + 1) * 128, :])

    # ball0 rides the gpsimd SWDGE queue (needed early, tiny); the vones
    # columns go LAST on the SP queue so their descriptor-heavy transfers
    # never jump ahead of the critical startup prefix on the serial engine.
    nc.gpsimd.dma_start(ball[0], ball_d[0:128, :])

    # ---- PE work units ----
    # fp8 DoubleRow 1.5-split: hi*hi + lo*hi + hi*lo (the lo*lo term is
    # ~1e-3-relative and dropped).  Each DoubleRow matmul contracts a pair of
    # 128-row k-blocks at 0.5 cycles/row, so a unit costs 12 mms x 128 cy
    # vs bf16's 4 x 512.
    SPLIT = ((0, 0), (1, 0), (0, 1))  # (x piece, w piece)

    def qk_unit(wn, bn, dest, m, n, mid=None, drain=None):
        pt = pj_tile()
        for nh in range(2):
            nsl = slice(n * CHUNK + nh * 256, n * CHUNK + (nh + 1) * 256)
            osl = slice(nh * 256, (nh + 1) * 256)
            for ti, (xi, wi) in enumerate(SPLIT):
                for j2 in range(CB // 2):
                    nc.tensor.matmul(
                        pt[:, osl],
                        lhsT=w[wn][wi][:, 2 * j2:2 * j2 + 2, m * 128:(m + 1) * 128],
                        rhs=xs8[xi][:, 2 * j2:2 * j2 + 2, nsl],
                        start=(ti == 0 and j2 == 0),
                        stop=(ti == 2 and j2 == 1),
                        perf_mode=DR,
                    )
            if mid is not None and nh == 0:
                mid()
        if drain is not None:
            drain(dest[m][:, n * CHUNK:(n + 1) * CHUNK], pt, bias[bn][m])
        else:
            nc.vector.tensor_scalar_add(
                dest[m][:, n * CHUNK:(n + 1) * CHUNK], pt[:, 0:512], bias[bn][m]
            )

    def v_unit(t2, mid=None):
        pt = pj_tile()
        tsl = slice(t2 * 128, (t2 + 1) * 128)
        for nh in range(2):
            osl = slice(nh * 256, (nh + 1) * 256)
            for ti, (xi, wi) in enumerate(SPLIT):
                for j2 in range(CB // 2):
                    nc.tensor.matmul(
                        pt[:, osl],
                        lhsT=xs8[xi][:, 2 * j2:2 * j2 + 2, tsl],
                        rhs=w["wvT"][wi][:, 2 * j2:2 * j2 + 2, osl],
                        start=(ti == 0 and j2 == 0),
                        stop=(ti == 2 and j2 == 1),
                        perf_mode=DR,
                    )
            if mid is not None and nh == 0:
                mid()
        nc.vector.tensor_tensor(
            v3[t2][:, :, 0:HD],
            pt[:, 0:512].rearrange("p (h d) -> p h d", d=HD),
            bv_bc.rearrange("p (h d) -> p h d", d=HD),
            ADD,
        )

    def o_unit(js, m, n, seed=False, final=False, pt=None, swdge=False,
               drain=None):
        # output projection partial over K blocks `js` (pairs), accumulated
        # in SBUF (seed carries the bias).  A final unit folds the SBUF
        # accumulator back into the PSUM group with an identity matmul (PE,
        # cheap) so the drain is a plain copy on whichever engine has slack.
        if pt is None:
            pt = pj_tile()
        if final:
            nc.tensor.matmul(pt[:, 0:512], lhsT=ident, rhs=outacc[m][n],
                             start=True, stop=False)
        for i, j in enumerate(js):
            nc.tensor.matmul(
                pt[:, 0:512],
                lhsT=w["woT"][j][:, m * 128:(m + 1) * 128],
                rhs=zT[j][:, n * CHUNK:(n + 1) * CHUNK],
                start=(not final and i == 0),
                stop=(i == len(js) - 1),
            )
        if final and drain is not False:
            (drain or nc.vector.tensor_copy)(
                outT[m][:, n * CHUNK:(n + 1) * CHUNK], pt[:, 0:512])
        elif seed:
            nc.vector.tensor_scalar_add(outacc[m][n], pt[:, 0:512], bias["bo"][m])
        else:
            nc.vector.tensor_tensor(outacc[m][n], pt[:, 0:512], outacc[m][n], ADD)

    # ---- token-major attention stream ----
    # Scores stay [keys, queries]; exp tiles E[t2, t1] then act as the
    # stationary operand of the ctx matmuls, so ctx lands token-major
    # [t1, dv] with all 128 output partitions active (the channel-major form
    # only fills 65).  Column 64 of each ctx group accumulates the softmax
    # denominator via the ones column in v.  After both heads of a pair are
    # normalized into a shared [t1, dv-pair] block, PE transposes flip it to
    # channel-major zT for the (unchanged) output projection.
    ESC = float(1.0 / np.sqrt(HD) / (WS * WS))
    GW = HD + 1            # ctx group width (64 dv + denominator)
    slots = {}

    def ctx_group(g):
        k = g // 7
        if k not in slots:
            slots[k] = ctx_slot_tile()
        c = (g % 7) * GW
        return slots[k][:, c:c + GW]

    E_store = {}
    # a few exps run on the DVE via corrected Schraudolph so the saturated
    # ACT exp chain (the kernel's spine) shortens; sites are picked away
    # from normalize/transpose steps so the DVE queue stays clear.
    # Corrected-Schraudolph DVE exp (see dve_exp) is numerically good
    # (0.3% rms) but measured net-negative at any site: its ~3.3us of
    # consecutive DVE ops delays the in-order drain queue that feeds the
    # PE stream.  Left disabled; would need per-step op interleaving.
    DVE_EXP = set()
    mask127 = np_pool.tile([128, 1024], I16, tag="m127", bufs=1, name="m127")
    nc.gpsimd.memset(mask127, 127)

    def dve_exp(E, sc):
        bs = np_pool.tile([128, 1024], I16, tag="xbs", bufs=1, name="xbs")
        mu = np_pool.tile([128, 1024], I16, tag="xmu", bufs=1, name="xmu")
        mC = np_pool.tile([128, 1024], MM_DT, tag="xmc", bufs=1, name="xmc")
        w = np_pool.tile([128, 1024], MM_DT, tag="xw", bufs=1, name="xw")
        nc.vector.tensor_scalar(bs, sc, SCH_A, 16256.0, MULT, ADD)
        nc.vector.tensor_tensor(mu, bs, mask127, BAND)
        nc.vector.tensor_scalar(mC, mu, SCH_K, None, MULT)
        nc.vector.scalar_tensor_tensor(w, mu, -128.0, mC, ADD, MULT)
        nc.vector.tensor_tensor(E.bitcast(I16), bs, w, ADD)

    def scores_exp(h, t2):
        hp, par = divmod(h, 2)
        rows = slice(par * 64, (par + 1) * 64)
        sc = sc_tile()
        for half in range(2):
            nc.tensor.matmul(
                sc[:, half * 512:(half + 1) * 512],
                lhsT=kT[hp][rows, t2 * 128:(t2 + 1) * 128],
                rhs=qT[hp][rows, half * 512:(half + 1) * 512],
                start=True, stop=True,
                tile_position=(par * 64, 0),
            )
        E = ep.tile([128, 1024], MM_DT, tag="E", name="E")
        if (h, t2) in DVE_EXP:
            dve_exp(E, sc)
        else:
            nc.scalar.activation(E, sc, EXP, scale=ESC)
        E_store[(h, t2)] = E

    def ctx_burst(h, s0, s1):
        # s-major: each ctx group's 8 accumulation matmuls run back-to-back
        # and the group closes before the next opens — interleaving open
        # accumulation groups within one PSUM bank corrupts all but the
        # last-started group (hardware pending-zero granularity).
        for s in range(s0, s1):
            grp = ctx_group(8 * h + s)
            for t2 in range(TB):
                nc.tensor.matmul(
                    grp,
                    lhsT=E_store[(h, t2)][:, s * 128:(s + 1) * 128],
                    rhs=v[t2][:, h * GW:(h + 1) * GW],
                    start=(t2 == 0), stop=(t2 == TB - 1),
                )
        if s1 == 8:
            for t2 in range(TB):
                E_store.pop((h, t2))

    ztoks = {}

    def ztok_tile(p):
        if p not in ztoks:
            ztoks[p] = np_pool.tile([128, S], MM_DT, tag="ztok", bufs=2,
                                    name="ztok")
        return ztoks[p]

    def norm_unit(h):
        # reciprocal of the 8 denominator columns, then one strided multiply
        # per slot-run into the pair's token-major block (per-partition
        # scalars: no gpsimd broadcast needed in this orientation).
        par = h % 2
        ztr = ztok_tile(h // 2).rearrange("p (s q) -> p s q", q=128)
        rs = np_pool.tile([128, 8], F32, tag="rs", bufs=2, name="rs")
        rsr = rs.rearrange("p (g o) -> p g o", o=1)
        g0 = 8 * h
        views = []
        b = g0
        while b < g0 + 8:
            e = min(g0 + 8, (b // 7 + 1) * 7)
            kr = slots[b // 7].rearrange("p (g c) -> p g c", c=GW)
            i0 = b % 7
            views.append((b - g0, e - b, kr[:, i0:i0 + e - b, :]))
            b = e
        for s0, n, kr in views:
            nc.vector.reciprocal(rsr[:, s0:s0 + n, :], kr[:, :, HD:HD + 1])
        for s0, n, kr in views:
            nc.vector.tensor_tensor(
                ztr[:, s0:s0 + n, par * 64:(par + 1) * 64],
                kr[:, :, 0:HD],
                rsr[:, s0:s0 + n, :].broadcast_to([128, n, HD]),
                MULT,
            )

    def tr_unit(p, half, head_only=False):
        # 4 PE transposes of [t1-slice, dv-pair] blocks into a bf16 view of a
        # pj rotation slot, drained as one [128, 512] copy into zT[p].
        # head_only: pair 3's odd head runs channel-major (65-partition ctx),
        # so only head 6's 64 dv columns go through the transpose — and since
        # cx7 owns the pj rotations by then, pair 3 borrows a ctx-ring bank.
        trv = (ctx_slot_tile() if head_only else pj_tile()).bitcast(MM_DT)
        ztp = ztok_tile(p)
        wdt = 64 if head_only else 128
        for sl in range(4):
            s = half * 4 + sl
            nc.tensor.matmul(
                trv[0:wdt, sl * 128:(sl + 1) * 128],
                lhsT=ztp[:, s * 128:s * 128 + wdt],
                rhs=ident, is_transpose=True,
            )
        nc.vector.tensor_copy(zT[p][0:wdt, half * 512:(half + 1) * 512],
                              trv[0:wdt, 0:512])

    # head 7 runs the channel-major 65-partition ctx form so its ctx matmuls
    # pipeline over t2 (only the last block waits on the final exp) and the
    # normalized rows land directly in zT[3][64:128] — this keeps the program
    # tail short.  cx7 comes from the last two pj rotations; no pj user may
    # follow until the output-projection finals (which scavenge sc tiles).
    cx7 = []

    def ctx65(t2):
        if not cx7:
            cx7.extend((pj_tile(), pj_tile()))
        for half in range(2):
            nc.tensor.matmul(
                cx7[half][0:HD + 1, :],
                lhsT=v[t2][:, 7 * GW:8 * GW],
                rhs=E_store[(7, t2)][:, half * 512:(half + 1) * 512],
                start=(t2 == 0), stop=(t2 == TB - 1),
            )
        if t2 == TB - 1:
            for tt in range(TB):
                E_store.pop((7, tt))

    # ---- emission schedule ----
    # Step sigma = 8h + t2.  Fixed slots by sigma%8: +4..+7 carry ctx batches
    # (h, t2-4); +0/+1 carry the previous head's late ctx batches (lag
    # shrinks to 2 for t2=7 so normalize lands early enough for the 14-slot
    # ctx ring); +2 normalizes the previous head.  tr units ride the +3/+4
    # slots after odd heads.  Everything else (projections, output-partial
    # units) fills the remaining slots.
    F = lambda f, *a, **k: (lambda: f(*a, **k))

    # startup: the minimal chain to the first exp is q00+k00 -> scores(0,0)
    # half A; q01 and v0 ride between the two half-exps.
    IDENT_F = mybir.ActivationFunctionType.Identity

    def act_drain(dest, pt, b):
        nc.scalar.activation(dest, pt[:, 0:512], IDENT_F, bias=b)

    qk_unit("wqT", "bq", qT, 0, 0)
    qk_unit("wkT", "bk", kT, 0, 0, drain=act_drain)
    sc0 = sc_tile()
    E0 = ep.tile([128, 1024], MM_DT, tag="E", name="E")
    nc.tensor.matmul(sc0[:, 0:512], lhsT=kT[0][0:64, 0:128],
                     rhs=qT[0][0:64, 0:512], start=True, stop=True,
                     tile_position=(0, 0))
    nc.scalar.activation(E0[:, 0:512], sc0[:, 0:512], EXP, scale=ESC)
    qk_unit("wqT", "bq", qT, 0, 1)
    # scores(0,1) half A needs only q00+k00 — its exp fills the ACT gap
    # while the B halves wait on q01's drain (x2-DMA-gated).
    sc1 = sc_tile()
    E1 = ep.tile([128, 1024], MM_DT, tag="E", name="E")
    nc.tensor.matmul(sc1[:, 0:512], lhsT=kT[0][0:64, 128:256],
                     rhs=qT[0][0:64, 0:512], start=True, stop=True,
                     tile_position=(0, 0))
    nc.scalar.activation(E1[:, 0:512], sc1[:, 0:512], EXP, scale=ESC)
    nc.tensor.matmul(sc0[:, 512:1024], lhsT=kT[0][0:64, 0:128],
                     rhs=qT[0][0:64, 512:1024], start=True, stop=True,
                     tile_position=(0, 0))
    nc.scalar.activation(E0[:, 512:1024], sc0[:, 512:1024], EXP, scale=ESC)
    nc.tensor.matmul(sc1[:, 512:1024], lhsT=kT[0][0:64, 128:256],
                     rhs=qT[0][0:64, 512:1024], start=True, stop=True,
                     tile_position=(0, 0))
    nc.scalar.activation(E1[:, 512:1024], sc1[:, 512:1024], EXP, scale=ESC)
    E_store[(0, 0)] = E0
    E_store[(0, 1)] = E1
    qk_unit("wkT", "bk", kT, 0, 1)

    fillers = {
        4: [F(v_unit, 0)], 5: [F(v_unit, 1), F(v_unit, 2)],
        6: [F(v_unit, 3), F(v_unit, 4)], 7: [F(v_unit, 5), F(v_unit, 6)],
        8: [F(v_unit, 7)],
        12: [F(qk_unit, "wqT", "bq", qT, 1, 0)],
        13: [F(qk_unit, "wkT", "bk", kT, 1, 0)],
        14: [F(qk_unit, "wqT", "bq", qT, 1, 1)],
        15: [F(qk_unit, "wkT", "bk", kT, 1, 1)],
        22: [F(tr_unit, 0, 0)],
        23: [F(tr_unit, 0, 1)],
        24: [F(qk_unit, "wqT", "bq", qT, 2, 0)],
        25: [F(qk_unit, "wkT", "bk", kT, 2, 0)],
        26: [F(qk_unit, "wqT", "bq", qT, 2, 1)],
        27: [F(qk_unit, "wkT", "bk", kT, 2, 1)],
        33: [F(qk_unit, "wqT", "bq", qT, 3, 0)],
        34: [F(qk_unit, "wkT", "bk", kT, 3, 0)],
        35: [F(qk_unit, "wqT", "bq", qT, 3, 1)],
        36: [F(qk_unit, "wkT", "bk", kT, 3, 1)],
        38: [F(tr_unit, 1, 0)],
        39: [F(tr_unit, 1, 1)],
        41: [F(o_unit, (0, 1), 0, 0, True)],
        42: [F(o_unit, (0, 1), 1, 0, True)],
        43: [F(o_unit, (0, 1), 2, 0, True)],
        44: [F(o_unit, (0, 1), 3, 0, True)],
        45: [F(o_unit, (0, 1), 0, 1, True)],
        46: [F(o_unit, (0, 1), 1, 1, True)],
        47: [F(o_unit, (0, 1), 2, 1, True)],
        48: [F(o_unit, (0, 1), 3, 1, True)],
        54: [F(tr_unit, 2, 0)],
        55: [F(tr_unit, 2, 1)],
        # pj users must all precede sig 60 where ctx65 claims cx7; pair 3's
        # transposes borrow the ctx ring instead.
        56: [F(o_unit, (2,), 0, 0), F(o_unit, (2,), 1, 0)],
        57: [F(o_unit, (2,), 2, 0), F(o_unit, (2,), 3, 0)],
        58: [F(o_unit, (2,), 0, 1), F(o_unit, (2,), 1, 1)],
        59: [F(o_unit, (2,), 2, 1), F(o_unit, (2,), 3, 1)],
        62: [F(tr_unit, 3, 0, True)],
        63: [F(tr_unit, 3, 1, True)],
    }

    for sig in range(1, 64):
        h, t2 = divmod(sig, 8)
        if sig != 1:          # (0,1) was emitted half-split in the startup
            scores_exp(h, t2)
        if h > 0 and 1 <= t2 <= 4:
            ctx_burst(h - 1, 2 * (t2 - 1), 2 * (t2 - 1) + 2)
        elif h > 0 and t2 == 5:
            norm_unit(h - 1)
        if h == 7 and t2 >= 4:
            ctx65(t2 - 4)
        for f in fillers.get(sig, ()):
            f()

    # ---- tail ----
    # head 7's trailing ctx65 blocks chase the last exps; its normalize goes
    # through the channel-major path (recip row + gpsimd broadcast) straight
    # into zT[3][64:128].  Keepers hold the PE p-state across the vector-side
    # latency; the finals scavenge sc tiles (pj holds cx7).
    ctx65(4)
    ctx65(5)
    ctx65(6)
    ctx65(7)
    # quarter-granular normalize pipeline: recip -> broadcast -> multiply
    # flows per [*, 256] chunk so the first zT quarters land early.
    rss, rbs = [], []
    for q in range(4):
        rs = np_pool.tile([1, 256], F32, tag="rs7", name="rs7")
        nc.vector.reciprocal(
            rs[0:1, :], cx7[q // 2][64:65, (q % 2) * 256:(q % 2 + 1) * 256])
        rss.append(rs)
        rb = np_pool.tile([64, 256], F32, tag="rb7", name="rb7")
        nc.gpsimd.partition_broadcast(rb, rs[0:1, :], 64)
        rbs.append(rb)
    # keepers: chained off the normalize stages so the PE p-state holds
    # across the whole PE-idle window (fp32 operands run 4 cy/row — good).
    warm = sc_tile()
    nc.tensor.matmul(warm[:, 0:512], lhsT=ident, rhs=qT[0][:, 0:512],
                     start=True, stop=True)
    nc.tensor.matmul(warm[0:64, 512:768], lhsT=bv_bc[0:64, 0:64],
                     rhs=rbs[0][:, 0:256], start=True, stop=True)
    nc.tensor.matmul(warm[0:64, 768:1024], lhsT=bv_bc[0:64, 0:64],
                     rhs=rbs[1][:, 0:256], start=True, stop=True)
    for q in range(4):
        nc.vector.tensor_tensor(
            zT[3][64:128, q * 256:(q + 1) * 256],
            cx7[q // 2][0:64, (q % 2) * 256:(q % 2 + 1) * 256], rbs[q], MULT)
    # finals: 2 per scavenged sc tile, drains alternating ACT/DVE, one merged
    # [128, 1024] output DMA per m-block (the transfer engine is serial, so
    # fewer, larger transfers shorten the issue pipeline).
    # finals: per-tile units (accumulator fold first inside o_unit), homes
    # chosen so WAR chains resolve during the normalize window.
    drains = (nc.scalar.copy, nc.vector.tensor_copy)
    homes = {0: None, 1: None, 2: "pj", 3: "warm"}
    for m in range(CB):
        if homes[m] == "pj":
            pts = [pj_tile() for _ in range(NCH)]
        elif homes[m] == "warm":
            pts = [warm[:, n * 512:(n + 1) * 512] for n in range(NCH)]
        else:
            sct = sc_tile()
            pts = [sct[:, n * 512:(n + 1) * 512] for n in range(NCH)]
        for n in range(NCH):
            o_unit((3,), m, n, final=True, pt=pts[n],
                   drain=drains[(2 * m + n) % 2])
        nc.sync.dma_start(out_d[m * 128:(m + 1) * 128, :], outT[m])


_NC_CACHE = None


def _get_nc():
    global _NC_CACHE
    if _NC_CACHE is None:
        _NC_CACHE = build_nc()
    return _NC_CACHE


def _split_f8(a, f8):
    """fp8 hi/lo split: a ~= hi + lo elementwise."""
    hi = a.astype(f8)
    lo = (a - hi.astype(np.float32)).astype(f8)
    return hi, lo


def _blk(a):
    """[512, N] -> [128, 4, N] -> [128, 4*N] (contraction-block-major)."""
    n = a.shape[1]
    return np.ascontiguousarray(
        a.reshape(CB, 128, n).transpose(1, 0, 2).reshape(128, CB * n)
    )


def _in_maps(x, Wq, bq, Wk, bk, Wv, bv, Wo, bo):
    x = np.ascontiguousarray(np.asarray(x, np.float32))
    bf16 = mybir.dt.np(MM_DT)
    f8 = mybir.dt.np(F8)
    # q/k/v weights carry a x16 scale (keeps W and its fp8 residual out of
    # the subnormal range); compensated by x16 biases, vones=16 (so the
    # softmax denominator scales identically) and the exp scale /256.
    wsplit = [_split_f8(np.asarray(W, np.float32).T * WS, f8) for W in (Wq, Wk, Wv)]
    w8 = np.concatenate([_blk(p) for hl in wsplit for p in hl], axis=1)
    base = {
        "bv_bc": np.ascontiguousarray(
            np.broadcast_to(np.asarray(bv, np.float32) * WS, (128, C))
        ),
        "w8": w8,
        "woT": np.ascontiguousarray(np.asarray(Wo, np.float32).T.astype(bf16)),
        "ball": np.ascontiguousarray(np.stack(
            [np.asarray(bq, np.float32) * WS, np.asarray(bk, np.float32) * WS,
             np.asarray(bo, np.float32)], axis=1)),
        "vones": np.full((128, 64), WS, bf16),
        "ident": np.eye(128, dtype=bf16),
    }

    def xmap(b):
        xsT = x[b].reshape(S, C).T  # [C, S] channel-major, f32
        hi, lo = _split_f8(xsT, f8)
        return np.concatenate([_blk(hi), _blk(lo)], axis=1)

    return [dict(base, x8=xmap(b)) for b in range(B)]


def _run(trace=False, **inputs):
    nc = _get_nc()
    maps = _in_maps(**inputs)
    res = run_bass_kernel_spmd(nc, maps, core_ids=list(range(B)), trace=trace)
    out = np.stack(
        [np.asarray(res.results[b]["out"]).reshape(C, HH, WW) for b in range(B)]
    ).astype(np.float32)
    return out, res


def kernel(**inputs):
    out, _ = _run(trace=False, **inputs)
    return out

